# revision 41
# baseline (speedup 1.0000x reference)
"""Trainium2 Bass kernel for a LongNet attention block.

Problem: x (1,48,256,256) -> patchify to 16384 tokens of dim 192 ->
4 segments of 4096 tokens -> q/k/v proj + LayerNorm each -> full
attention within each segment -> un-patchify.

Sharding: 2 cores per segment (8 cores, 4 segments). Each core computes
attention for 2048 queries against its segment's full 4096 keys/values.
Softmax is key-order invariant, so the host permutes each core's token
columns so its query half is always columns 0:2048 -> one SPMD program.

Fast path (gamma=1, beta=0), fp16 operands + fp8 DoubleRow out-stage:
  1. Natural k/q/v projections (fp16 in, f32 PSUM) exist only to feed
     the LN sum-of-square stats; bias via a ones-row, mean-centering
     folded into the weights on the host. Stats are BATCHED four
     chunks per op: one ScalarE Square over a [128,~4D] group tile +
     one DVE tensor_reduce(axis=X) -> four per-token sums. V naturals
     are interleaved as PE filler behind the K/Q critical chain.
  2. kT and qT are projected DIRECTLY in transposed layout (weights as
     lhsT, x as rhs) - no PE transposes. The q dims 128:192 use a
     host-duplicated weight so the two K=64 row-tile copies land
     pre-duplicated at partitions 0:64/64:128.
  3. LN rsqrt scales (exp(-v/2+1/2) seed + 2 Newton steps) are
     PE-transposed to rows, broadcast to [128,512] tiles via one-hot
     K-matmuls, and fused into the PSUM->SBUF eviction multiplies.
     k-hat is pre-scaled by C*rsqrt(ssq_k/D+eps) so the exp needs only
     a constant bias, enabling [128,1024] two-chunk exps.
  4. Out-stage in fp8e4 DoubleRow: chunk pairs (256 keys) per matmul,
     software-pipelined one pair behind scores/exp so the PE queue
     never stalls on the exp semaphore. p-hat = fp8(p * 2^psi[key])
     via the exp bias AP; v-hat = fp8(v * r_v * 2^-psi); the v
     ones-column holds fp8-exact 2^-psi. Products are exactly
     compensated; the rotating quantization phase decorrelates RNE
     rounding across keys in diffuse-softmax rows. v-hat staging
     drains on DVE inside attention slab 0.
  5. outT accumulates in PSUM; ones-column gives the denominator row.
     Host divides and transposes.
"""

import contextlib

import numpy as np
import ml_dtypes

import concourse.bacc as bacc
import concourse.mybir as mybir
import concourse.tile as tile
from concourse.bass_utils import run_bass_kernel_spmd

WS = 2
C = 48
IMG = 256
NS = IMG // WS          # 128
D = C * WS * WS         # 192
S = NS * NS             # 16384
SEG = 4096
G = S // SEG            # 4 segments
NQ = SEG // 2           # 2048 queries per core
NCORES = 8
EPS = 1e-5
SCALE_C = float(D) ** -0.5
SLAB = 512
NKC = SEG // 128        # 32 key chunks
NQC = NQ // 128         # 16 query chunks
NPAIR = NKC // 2        # 16 key-chunk pairs
NSL = NQ // SLAB        # 4 query slabs
NT = NKC // 4           # 8 key s-tiles of 512
VW = 208                # fp8 v pair tile inner width (193 used, 16B align)
SHIFT = 0.7             # exp bias shift: p = exp(s - SHIFT + psi*ln2)

F32 = mybir.dt.float32
F16 = mybir.dt.float16
BF16 = mybir.dt.bfloat16
FP8 = mybir.dt.float8e4
FT = mybir.ActivationFunctionType
OP = mybir.AluOpType
DRM = mybir.MatmulPerfMode.DoubleRow

# fp8-exact dither weights w = 2^-psi (3-mantissa-bit exact values)
_WPH = np.array([1.0, 0.9375, 0.875, 0.8125, 0.75], np.float32)
_WCOL = _WPH[np.arange(128) % 5]                      # per-partition w

_PROGRAM_CACHE = {}


def _build_program_fast():
    nc = bacc.Bacc(
        "TRN2",
        target_bir_lowering=False,
        debug=False,
        enable_asserts=False,
    )
    xa = nc.dram_tensor("xa", [128, SEG], F16, kind="ExternalInput").ap()
    xb = nc.dram_tensor("xb", [128, SEG], F16, kind="ExternalInput").ap()
    wa = nc.dram_tensor("wa", [128, 3 * D], F16, kind="ExternalInput").ap()
    wb = nc.dram_tensor("wb", [128, 3 * D], F16, kind="ExternalInput").ap()
    wka = nc.dram_tensor("wka", [128, D], F16, kind="ExternalInput").ap()
    wkb = nc.dram_tensor("wkb", [128, D], F16, kind="ExternalInput").ap()
    # wqd: q dims 128:192 duplicated twice along cols (so the transposed
    # q1 projection lands pre-duplicated at partitions 0:64 and 64:128).
    # cols 0:128 contract xa, cols 128:256 contract xb (incl bias row).
    wqd = nc.dram_tensor("wqd", [128, 256], F16, kind="ExternalInput").ap()
    idn = nc.dram_tensor("idn", [128, 128], F16, kind="ExternalInput").ap()
    dithb = nc.dram_tensor("dithb", [128, 1], F32, kind="ExternalInput").ap()
    dithw = nc.dram_tensor("dithw", [128, 1], F32, kind="ExternalInput").ap()
    ones8 = nc.dram_tensor("ones8", [128, 1], FP8, kind="ExternalInput").ap()
    selbc = nc.dram_tensor("selbc", [32, SEG], F16, kind="ExternalInput").ap()
    outa = nc.dram_tensor("outa", [128, NQ], F32, kind="ExternalOutput").ap()
    outb = nc.dram_tensor("outb", [65, NQ], F32, kind="ExternalOutput").ap()

    with tile.TileContext(nc) as tc:
        with contextlib.ExitStack() as stk:
            const = stk.enter_context(tc.tile_pool(name="const", bufs=1))
            persist = stk.enter_context(tc.tile_pool(name="persist", bufs=1))
            ln_sb = stk.enter_context(tc.tile_pool(name="ln_sb", bufs=4))
            smalls = stk.enter_context(tc.tile_pool(name="smalls", bufs=4))
            pt_pool = stk.enter_context(tc.tile_pool(name="pt_pool", bufs=4))
            ev = stk.enter_context(tc.tile_pool(name="ev", bufs=4))

            # weights first so the first projection matmuls can start
            # as soon as the first token tile lands
            wa_s = const.tile([128, 3 * D], F16)
            nc.sync.dma_start(wa_s[:, D:2 * D], wa[:, D:2 * D])
            xabuf = const.tile([128, SEG], F16, name="xabuf")
            xbbuf = const.tile([128, SEG], F16, name="xbbuf")
            wb_s = const.tile([128, 3 * D], F16)
            nc.sync.dma_start(wb_s[:, D:2 * D], wb[:, D:2 * D])
            nc.sync.dma_start(xabuf[:, 0:1024], xa[:, 0:1024])
            nc.sync.dma_start(xbbuf[:, 0:1024], xb[:, 0:1024])
            nc.sync.dma_start(wa_s[:, 0:D], wa[:, 0:D])
            nc.sync.dma_start(wa_s[:, 2 * D:3 * D], wa[:, 2 * D:3 * D])
            nc.sync.dma_start(wb_s[:, 0:D], wb[:, 0:D])
            nc.sync.dma_start(wb_s[:, 2 * D:3 * D], wb[:, 2 * D:3 * D])
            wka_s = const.tile([128, D], F16)
            nc.sync.dma_start(wka_s, wka)
            wkb_s = const.tile([128, D], F16)
            nc.sync.dma_start(wkb_s, wkb)
            wqd_s = const.tile([128, 256], F16)
            nc.sync.dma_start(wqd_s, wqd)
            idn_s = const.tile([128, 128], F16)
            nc.sync.dma_start(idn_s, idn)
            # one batched DMA per buffer for the tail (Sync dispatch is
            # ~600ns per DMA instruction; fewer, bigger transfers)
            nc.sync.dma_start(xabuf[:, 1024:SEG], xa[:, 1024:SEG])
            nc.sync.dma_start(xbbuf[:, 1024:SEG], xb[:, 1024:SEG])
            xat = [xabuf[:, t * 512:(t + 1) * 512] for t in range(NT)]
            xbt = [xbbuf[:, t * 512:(t + 1) * 512] for t in range(NT)]
            dithb_s = const.tile([128, 1], F32)
            nc.sync.dma_start(dithb_s, dithb)
            dithw_s = const.tile([128, 1], F32)
            nc.sync.dma_start(dithw_s, dithw)
            ones8_s = const.tile([128, 1], FP8)
            nc.sync.dma_start(ones8_s, ones8)
            halfc = const.tile([128, 1], F32)
            nc.gpsimd.memset(halfc, 0.5)

            # persistent state
            qT0s = [persist.tile([128, SLAB], F16, name=f"qT0s{s}")
                    for s in range(NSL)]
            qT1s = [persist.tile([128, SLAB], F16, name=f"qT1s{s}")
                    for s in range(NSL)]
            kT0t = [persist.tile([128, 512], F16, name=f"kT0t{t}")
                    for t in range(NT)]
            # pair layout: block b in {0,1} holds chunks (4t+2b, 4t+2b+1)
            # at rows 0:64 / 64:128 (k-hat dims 128:192)
            kT1t = [persist.tile([128, 256], F16, name=f"kT1t{t}")
                    for t in range(NT)]
            # fp8 v pair tiles: [key-in-chunk, which-chunk, 192 dims+ones]
            vat8 = [persist.tile([128, 2, VW], FP8, name=f"vat8{j}")
                    for j in range(NPAIR)]
            cpreV = [persist.tile([128, 4 * D], F16, name=f"cpreVg{g}")
                     for g in range(NKC // 4)]
            rowk = persist.tile([32, 128], F16)
            rowq = persist.tile([16, 128], F16)
            selbc_s = persist.tile([32, SEG], F16)
            nc.sync.dma_start(selbc_s, selbc)
            ssqQ = persist.tile([128, NQC], F32)
            ssqKV = persist.tile([128, 2 * NKC], F32)  # k: 0:32, v: 32:64
            rQ = persist.tile([128, NQC], F32)
            rKV = persist.tile([128, 2 * NKC], F32)
            rkc16 = persist.tile([128, NKC], F16)   # f16(C / std_k) columns
            rqc16 = persist.tile([128, NQC], F16)   # f16(r_q) columns
            rvd = persist.tile([128, NKC], F32)     # r_v * 2^-psi columns
            for j in range(NPAIR):
                nc.vector.tensor_copy(vat8[j][:, 0, 192:193], ones8_s)
                nc.vector.tensor_copy(vat8[j][:, 1, 192:193], ones8_s)

            # ---- batched rsqrt: r = rsqrt(ssq/D + eps) ----
            def batched_r(ssq_t, r_t, w):
                vv = smalls.tile([128, 2 * NKC], F32, name="vv")
                nc.vector.tensor_scalar(vv[:, 0:w], ssq_t[:, 0:w],
                                        1.0 / D, EPS, OP.mult, OP.add)
                nc.scalar.activation(r_t[:, 0:w], vv[:, 0:w], FT.Exp,
                                     scale=-0.5, bias=halfc)
                hv = smalls.tile([128, 2 * NKC], F32, name="hv")
                nc.vector.tensor_scalar(hv[:, 0:w], vv[:, 0:w], -0.5,
                                        None, OP.mult)
                cur = r_t
                for it in range(2):
                    b = smalls.tile([128, 2 * NKC], F32, name=f"nb{it}")
                    nc.vector.tensor_tensor(b[:, 0:w], cur[:, 0:w],
                                            cur[:, 0:w], OP.mult)
                    t2 = smalls.tile([128, 2 * NKC], F32, name=f"nt{it}")
                    nc.vector.scalar_tensor_tensor(
                        t2[:, 0:w], b[:, 0:w], 1.0, hv[:, 0:w],
                        OP.mult, OP.mult)
                    nxt = r_t if it == 1 else smalls.tile(
                        [128, 2 * NKC], F32, name=f"nr{it}")
                    nc.vector.scalar_tensor_tensor(
                        nxt[:, 0:w], t2[:, 0:w], 1.5, cur[:, 0:w],
                        OP.add, OP.mult)
                    cur = nxt

            def kt_mats(pool, t):
                kp0 = pool.tile([128, 512], F32, name="kp0")
                nc.tensor.matmul(kp0, lhsT=wka_s[:, 0:128], rhs=xat[t],
                                 start=True, stop=False)
                nc.tensor.matmul(kp0, lhsT=wkb_s[:, 0:128], rhs=xbt[t],
                                 start=False, stop=True)
                kp1 = pool.tile([128, 512], F32, name="kp1")
                for j in range(4):
                    r0 = (j % 2) * 64
                    csl = slice(j * 128, (j + 1) * 128)
                    nc.tensor.matmul(kp1[r0:r0 + 64, csl],
                                     lhsT=wka_s[:, 128:192],
                                     rhs=xat[t][:, csl],
                                     start=True, stop=False)
                    nc.tensor.matmul(kp1[r0:r0 + 64, csl],
                                     lhsT=wkb_s[:, 128:192],
                                     rhs=xbt[t][:, csl],
                                     start=False, stop=True)
                return kp0, kp1

            # ---- Phase A: K naturals, group-4 batched stats ----
            # ---- Phase B: Q + V naturals interleaved (PE filler) ----
            NG = 4                       # chunks per stats group
            GD = NG * D

            with tc.tile_pool(name="paq", bufs=3, space="PSUM") as paq, \
                 tc.tile_pool(name="pa_tr", bufs=1, space="PSUM") as ptr:

                # paired reduces: consecutive groups of a stream share one
                # sqg2 tile so ONE DVE tensor_reduce covers 8 chunks
                # (DVE is the pre-attention wall at ~99% busy)
                sq_pend = {}

                def nat_group(g, wcol, ssq_t, col0, stage_to=None, key="k"):
                    """Project chunks 4g..4g+3 (N=D each) into one group
                    tile (2 chunks per PSUM bank, 384:512 padded); batched
                    square; one paired reduce per two groups."""
                    raw4 = paq.tile([128, 1024], F32, name="raw4")
                    for i in range(NG):
                        c = NG * g + i
                        jsl = slice((c % 4) * 128, (c % 4 + 1) * 128)
                        o = (i // 2) * 512 + (i % 2) * D
                        dst = raw4[:, o:o + D]
                        nc.tensor.matmul(dst, lhsT=xat[c // 4][:, jsl],
                                         rhs=wa_s[:, wcol:wcol + D],
                                         start=True, stop=False)
                        nc.tensor.matmul(dst, lhsT=xbt[c // 4][:, jsl],
                                         rhs=wb_s[:, wcol:wcol + D],
                                         start=False, stop=True)
                    rawv = raw4.rearrange("p (b x) -> p b x", b=2)[:, :, 0:2 * D]
                    if g % 2 == 0:
                        sqg2 = ln_sb.tile([128, 2 * GD], F16, name="sqg")
                        sq_pend[key] = sqg2
                    else:
                        sqg2 = sq_pend.pop(key)
                    half = sqg2[:, (g % 2) * GD:(g % 2) * GD + GD]
                    if stage_to is not None:
                        # alternate the staging cast DVE / ScalarE
                        st = stage_to.rearrange("p (b x) -> p b x", b=2)
                        if g % 2 == 0:
                            nc.vector.tensor_copy(st, rawv)
                        else:
                            nc.scalar.copy(st, rawv)
                        if g >= 6:
                            # tail groups: square on DVE so ScalarE's
                            # backlog releases the PSUM banks sooner
                            # (the kT matmuls wait on them; >3.4us PE
                            # idle re-throttles the HAM clock gate)
                            nc.vector.scalar_tensor_tensor(
                                half, stage_to, 1.0, stage_to,
                                OP.mult, OP.mult)
                        else:
                            nc.scalar.activation(half, stage_to, FT.Square)
                    else:
                        nc.scalar.activation(half, rawv, FT.Square)
                    if g % 2 == 1:
                        c0 = col0 + NG * (g - 1)
                        nc.vector.tensor_reduce(
                            ssq_t[:, c0:c0 + 2 * NG],
                            sqg2.rearrange("p (a d) -> p a d", a=2 * NG),
                            mybir.AxisListType.X, OP.add)

                for g in range(NKC // NG):
                    nat_group(g, D, ssqKV, 0)

                batched_r(ssqKV[:, 0:NKC], rKV[:, 0:NKC], NKC)
                nc.vector.tensor_scalar(rkc16, rKV[:, 0:NKC], SCALE_C,
                                        None, OP.mult)
                nat_group(0, 2 * D, ssqKV, NKC, stage_to=cpreV[0], key="v")
                nat_group(1, 2 * D, ssqKV, NKC, stage_to=cpreV[1], key="v")
                trp = ptr.tile([128, 128], F16, name="trp")
                nc.tensor.transpose(trp[0:32, :], rkc16, idn_s)
                nc.vector.tensor_copy(rowk, trp[0:32, :])

                vg = 2
                for g in range(NQC // NG):
                    nat_group(g, 0, ssqQ, 0, key="q")
                    if vg < 6:
                        nat_group(vg, 2 * D, ssqKV, NKC,
                                  stage_to=cpreV[vg], key="v")
                        vg += 1
                batched_r(ssqQ, rQ, NQC)
                nc.vector.tensor_copy(rqc16, rQ)
                trq = ptr.tile([128, 128], F16, name="trp")
                nc.tensor.transpose(trq[0:16, :], rqc16, idn_s)
                nc.vector.tensor_copy(rowq, trq[0:16, :])
                for g in range(vg, NKC // NG):
                    nat_group(g, 2 * D, ssqKV, NKC, stage_to=cpreV[g], key="v")
                batched_r(ssqKV[:, NKC:2 * NKC], rKV[:, NKC:2 * NKC], NKC)
                nc.vector.tensor_scalar(rvd, rKV[:, NKC:2 * NKC], dithw_s,
                                        None, OP.mult)

            # ---- transposed K/Q projections with fused-scale evicts ----
            with tc.tile_pool(name="pa_kt", bufs=3, space="PSUM") as pa_kt, \
                 tc.tile_pool(name="pa_bc", bufs=2, space="PSUM") as pa_bc:
                bc_ctr = [0]

                def bc_dma(row_t, r0, nrow=32):
                    bcp = pa_bc.tile([128, 512], F32, name="bcp")
                    for j in range(4):
                        r = r0 + j
                        nc.tensor.matmul(
                            bcp[:, j * 128:(j + 1) * 128],
                            lhsT=selbc_s[0:nrow, r * 128:(r + 1) * 128],
                            rhs=row_t,
                            start=True, stop=True)
                    bcs = ln_sb.tile([128, 512], F16, name="bcs")
                    # ScalarE is idle during kT staging; DVE is the wall
                    nc.scalar.copy(bcs, bcp)
                    bc_ctr[0] += 1
                    return bcs

                def kt_evict(t, kp0, kp1, bcs):
                    nc.vector.tensor_tensor(kT0t[t], kp0, bcs, OP.mult)
                    kp1v = kp1.rearrange("p (a b c) -> p a b c", a=2, b=2)
                    bcsv = bcs.rearrange("p (a b c) -> p a b c", a=2, b=2)
                    k1v = [kT1t[t][0:64].rearrange("p (a c) -> p a c", a=2),
                           kT1t[t][64:128].rearrange("p (a c) -> p a c", a=2)]
                    nc.vector.tensor_tensor(
                        k1v[0], kp1v[0:64, :, 0, :], bcsv[0:64, :, 0, :],
                        OP.mult)
                    nc.vector.tensor_tensor(
                        k1v[1], kp1v[64:128, :, 1, :], bcsv[64:128, :, 1, :],
                        OP.mult)

                bcs_next = bc_dma(rowk, 0, 32)
                for t in range(NT):
                    bcs = bcs_next
                    if t + 1 < NT:
                        bcs_next = bc_dma(rowk, 4 * (t + 1), 32)
                    kp0, kp1 = kt_mats(pa_kt, t)
                    kt_evict(t, kp0, kp1, bcs)

                for s in range(NSL):
                    bcqs = bc_dma(rowq, 4 * s, 16)
                    qp0 = pa_kt.tile([128, 512], F32, name="kp0")
                    nc.tensor.matmul(qp0, lhsT=wa_s[:, 0:128], rhs=xat[s],
                                     start=True, stop=False)
                    nc.tensor.matmul(qp0, lhsT=wb_s[:, 0:128], rhs=xbt[s],
                                     start=False, stop=True)
                    qp1 = pa_kt.tile([128, 512], F32, name="kp1")
                    nc.tensor.matmul(qp1, lhsT=wqd_s[:, 0:128], rhs=xat[s],
                                     start=True, stop=False)
                    nc.tensor.matmul(qp1, lhsT=wqd_s[:, 128:256], rhs=xbt[s],
                                     start=False, stop=True)
                    nc.vector.tensor_tensor(qT0s[s], qp0, bcqs, OP.mult)
                    nc.vector.tensor_tensor(qT1s[s], qp1, bcqs, OP.mult)

            # ---- attention: 4 slabs x 16 chunk pairs ----
            # The out-stage is software-pipelined one pair behind the
            # scores/exp so the PE queue never stalls on the exp sem.
            # v-finish ops interleave into slab 0 (DVE is idle here).
            def v_fin(c):
                nc.vector.tensor_scalar(
                    vat8[c // 2][:, c % 2, 0:D],
                    cpreV[c // 4][:, (c % 4) * D:(c % 4 + 1) * D],
                    rvd[:, c:c + 1], None, OP.mult)

            v_fin(0)
            v_fin(1)
            with tc.tile_pool(name="pcs_sc", bufs=3, space="PSUM") as pcs_sc, \
                 tc.tile_pool(name="pcs_oa", bufs=1, space="PSUM") as pcs_oa, \
                 tc.tile_pool(name="pcs_ob", bufs=1, space="PSUM") as pcs_ob:
                for s in range(NSL):
                    qsl = slice(s * SLAB, (s + 1) * SLAB)
                    oA = pcs_oa.tile([128, SLAB], F32, name="oA")
                    oB = pcs_ob.tile([65, SLAB], F32, name="oB")

                    def out_stage(j, pt):
                        pp = pt.rearrange("p (a b) -> p a b", a=2)
                        nc.tensor.matmul(oA, lhsT=vat8[j][:, :, 0:128],
                                         rhs=pp, start=(j == 0),
                                         stop=(j == NPAIR - 1),
                                         perf_mode=DRM)
                        nc.tensor.matmul(oB, lhsT=vat8[j][:, :, 128:193],
                                         rhs=pp, start=(j == 0),
                                         stop=(j == NPAIR - 1),
                                         perf_mode=DRM)

                    pt_prev = None
                    for pr in range(NPAIR):
                        c0 = 2 * pr
                        t = c0 // 4
                        pb = (c0 % 4) // 2   # pair block within kT1t[t]
                        sct = pcs_sc.tile([128, 1024], F32, name="sct")
                        for h in range(2):
                            c = c0 + h
                            jsl = slice((c % 4) * 128, (c % 4 + 1) * 128)
                            ssl = slice(h * 512, (h + 1) * 512)
                            nc.tensor.matmul(sct[:, ssl],
                                             lhsT=kT0t[t][:, jsl],
                                             rhs=qT0s[s],
                                             start=True, stop=False)
                        # second halves: two concurrent K=64 row tiles
                        psl = slice(pb * 128, (pb + 1) * 128)
                        nc.tensor.matmul(sct[:, 0:512],
                                         lhsT=kT1t[t][0:64, psl],
                                         rhs=qT1s[s][0:64, :],
                                         start=False, stop=True)
                        nc.tensor.matmul(sct[:, 512:1024],
                                         lhsT=kT1t[t][64:128, psl],
                                         rhs=qT1s[s][64:128, :],
                                         start=False, stop=True)
                        pt = pt_pool.tile([128, 1024], FP8, name="pt")
                        nc.scalar.activation(pt, sct, FT.Exp, bias=dithb_s)
                        if s == 0 and 2 * pr + 3 < NKC:
                            v_fin(2 * pr + 2)
                            v_fin(2 * pr + 3)
                        if pt_prev is not None:
                            out_stage(pr - 1, pt_prev)
                        pt_prev = pt
                    out_stage(NPAIR - 1, pt_prev)
                    ea = ev.tile([128, SLAB], F32, name="ea")
                    nc.vector.tensor_copy(ea, oA)
                    eb = ev.tile([65, SLAB], F32, name="eb")
                    nc.vector.tensor_copy(eb, oB)
                    nc.sync.dma_start(outa[:, qsl], ea)
                    nc.sync.dma_start(outb[:, qsl], eb)

    nc.compile()
    return nc


def _build_program_legacy(general_gb: bool):
    """Baseline bf16 kernel (handles general gamma/beta)."""
    nc = bacc.Bacc(
        "TRN2",
        target_bir_lowering=False,
        debug=False,
        enable_asserts=False,
    )
    xa = nc.dram_tensor("xa", [128, SEG], BF16, kind="ExternalInput").ap()
    xb = nc.dram_tensor("xb", [128, SEG], BF16, kind="ExternalInput").ap()
    wa = nc.dram_tensor("wa", [128, 3 * D], BF16, kind="ExternalInput").ap()
    wb = nc.dram_tensor("wb", [128, 3 * D], BF16, kind="ExternalInput").ap()
    wka = nc.dram_tensor("wka", [128, D], BF16, kind="ExternalInput").ap()
    wkb = nc.dram_tensor("wkb", [128, D], BF16, kind="ExternalInput").ap()
    idn = nc.dram_tensor("idn", [128, 128], BF16, kind="ExternalInput").ap()
    if general_gb:
        gcol = nc.dram_tensor("gcol", [D, 1], F32, kind="ExternalInput").ap()
        bcol = nc.dram_tensor("bcol", [D, 1], F32, kind="ExternalInput").ap()
        gbc = nc.dram_tensor("gbc", [128, D], F32, kind="ExternalInput").ap()
        bbc = nc.dram_tensor("bbc", [128, D], F32, kind="ExternalInput").ap()
    outa = nc.dram_tensor("outa", [128, NQ], F32, kind="ExternalOutput").ap()
    outb = nc.dram_tensor("outb", [65, NQ], F32, kind="ExternalOutput").ap()

    NSLl = NQ // SLAB
    VWl = 200

    with tile.TileContext(nc) as tc:
        with contextlib.ExitStack() as stk:
            const = stk.enter_context(tc.tile_pool(name="const", bufs=1))
            persist = stk.enter_context(tc.tile_pool(name="persist", bufs=1))
            ln_sb = stk.enter_context(tc.tile_pool(name="ln_sb", bufs=4))
            smalls = stk.enter_context(tc.tile_pool(name="smalls", bufs=4))
            pt_pool = stk.enter_context(tc.tile_pool(name="pt_pool", bufs=4))
            ev = stk.enter_context(tc.tile_pool(name="ev", bufs=4))

            xat = [const.tile([128, 512], BF16, name=f"xat{t}")
                   for t in range(NKC // 4)]
            xbt = [const.tile([128, 512], BF16, name=f"xbt{t}")
                   for t in range(NKC // 4)]
            for t in range(NKC // 4):
                tsl = slice(t * 512, (t + 1) * 512)
                nc.sync.dma_start(xat[t], xa[:, tsl])
                nc.sync.dma_start(xbt[t], xb[:, tsl])
            wa_s = const.tile([128, 3 * D], BF16)
            nc.sync.dma_start(wa_s, wa)
            wb_s = const.tile([128, 3 * D], BF16)
            nc.sync.dma_start(wb_s, wb)
            wka_s = const.tile([128, D], BF16)
            nc.sync.dma_start(wka_s, wka)
            wkb_s = const.tile([128, D], BF16)
            nc.sync.dma_start(wkb_s, wkb)
            idn_s = const.tile([128, 128], BF16)
            nc.sync.dma_start(idn_s, idn)
            epsc = const.tile([128, 1], F32)
            nc.gpsimd.memset(epsc, EPS)
            halfc = const.tile([128, 1], F32)
            nc.gpsimd.memset(halfc, 0.5)
            if general_gb:
                gca = const.tile([128, 1], F32)
                nc.sync.dma_start(gca, gcol[0:128])
                gcb = const.tile([64, 1], F32)
                nc.sync.dma_start(gcb, gcol[128:192])
                bca = const.tile([128, 1], F32)
                nc.sync.dma_start(bca, bcol[0:128])
                bcb = const.tile([64, 1], F32)
                nc.sync.dma_start(bcb, bcol[128:192])
                gbc_s = const.tile([128, D], F32)
                nc.sync.dma_start(gbc_s, gbc)
                bbc_s = const.tile([128, D], F32)
                nc.sync.dma_start(bbc_s, bbc)

            qT0s = [persist.tile([128, SLAB], BF16, name=f"qT0s{s}")
                    for s in range(NSLl)]
            qT1s = [persist.tile([128, SLAB], BF16, name=f"qT1s{s}")
                    for s in range(NSLl)]
            kT0t = [persist.tile([128, 512], BF16, name=f"kT0t{t}")
                    for t in range(NT)]
            kT1t = [persist.tile([128, 512], BF16, name=f"kT1t{t}")
                    for t in range(NT)]
            vatc = [persist.tile([128, VWl], BF16, name=f"vatc{c}")
                    for c in range(NKC)]
            cpreQ = [persist.tile([128, D], BF16, name=f"cpreQ{c}")
                     for c in range(NQC)]
            cpreV = [persist.tile([128, D], BF16, name=f"cpreV{c}")
                     for c in range(NKC)]
            cpreK = ([persist.tile([128, D], BF16, name=f"cpreK{c}")
                      for c in range(NKC)] if general_gb else None)
            ssqQ = persist.tile([128, NQC], F32)
            ssqKV = persist.tile([128, 2 * NKC], F32)
            rQ = persist.tile([128, NQC], F32)
            rKV = persist.tile([128, 2 * NKC], F32)
            rkc = persist.tile([128, NKC], F32)
            for s in range(NSLl):
                nc.gpsimd.memset(qT1s[s][64:128, :], 0.0)
            for t in range(NT):
                nc.gpsimd.memset(kT1t[t][64:128, :], 0.0)
            for c in range(NKC):
                nc.gpsimd.memset(vatc[c][:, 192:193], 1.0)

            def kt_proj(pool, t):
                kp0 = pool.tile([128, 512], F32, name="kp0")
                nc.tensor.matmul(kp0, lhsT=wka_s[:, 0:128], rhs=xat[t],
                                 start=True, stop=False)
                nc.tensor.matmul(kp0, lhsT=wkb_s[:, 0:128], rhs=xbt[t],
                                 start=False, stop=True)
                kp1 = pool.tile([64, 512], F32, name="kp1")
                nc.tensor.matmul(kp1, lhsT=wka_s[:, 128:192], rhs=xat[t],
                                 start=True, stop=False)
                nc.tensor.matmul(kp1, lhsT=wkb_s[:, 128:192], rhs=xbt[t],
                                 start=False, stop=True)
                nc.vector.tensor_copy(kT0t[t], kp0)
                nc.vector.tensor_copy(kT1t[t][0:64, :], kp1)

            with tc.tile_pool(name="pa_raw", bufs=3, space="PSUM") as pa_raw, \
                 tc.tile_pool(name="pa_kt", bufs=1, space="PSUM") as pa_kt:
                for c in range(NQC):
                    jsl = slice((c % 4) * 128, (c % 4 + 1) * 128)
                    raw = pa_raw.tile([128, D], F32, name="rawQ")
                    nc.tensor.matmul(raw, lhsT=xat[c // 4][:, jsl],
                                     rhs=wa_s[:, 0:D], start=True, stop=False)
                    nc.tensor.matmul(raw, lhsT=xbt[c // 4][:, jsl],
                                     rhs=wb_s[:, 0:D], start=False, stop=True)
                    nc.vector.tensor_copy(cpreQ[c], raw)
                    sqd = ln_sb.tile([128, D], BF16, name="sqd")
                    nc.scalar.activation(sqd, raw, FT.Square,
                                         accum_out=ssqQ[:, c:c + 1])
                    if not general_gb and c % 2 == 1:
                        kt_proj(pa_kt, c // 2)
                for c in range(NKC):
                    jsl = slice((c % 4) * 128, (c % 4 + 1) * 128)
                    raw = pa_raw.tile([128, 2 * D], F32, name="rawKV")
                    nc.tensor.matmul(raw, lhsT=xat[c // 4][:, jsl],
                                     rhs=wa_s[:, D:3 * D],
                                     start=True, stop=False)
                    nc.tensor.matmul(raw, lhsT=xbt[c // 4][:, jsl],
                                     rhs=wb_s[:, D:3 * D],
                                     start=False, stop=True)
                    sqd = ln_sb.tile([128, D], BF16, name="sqd")
                    nc.scalar.activation(sqd, raw[:, 0:D], FT.Square,
                                         accum_out=ssqKV[:, c:c + 1])
                    if general_gb:
                        nc.vector.tensor_copy(cpreK[c], raw[:, 0:D])
                    nc.vector.tensor_copy(cpreV[c], raw[:, D:2 * D])
                    if c % 2 == 0:
                        sqd2 = ln_sb.tile([128, D], BF16, name="sqd2")
                        nc.vector.scalar_tensor_tensor(
                            sqd2, cpreV[c], 1.0, cpreV[c], OP.mult, OP.mult,
                            accum_out=ssqKV[:, NKC + c:NKC + c + 1])
                    else:
                        sqd2 = ln_sb.tile([128, D], BF16, name="sqd2")
                        nc.scalar.activation(
                            sqd2, raw[:, D:2 * D], FT.Square,
                            accum_out=ssqKV[:, NKC + c:NKC + c + 1])

            def batched_r(ssq_t, r_t, w):
                vv = smalls.tile([128, 2 * NKC], F32, name="vv")
                nc.vector.tensor_scalar(vv[:, 0:w], ssq_t[:, 0:w], 1.0 / D,
                                        EPS, OP.mult, OP.add)
                nc.scalar.activation(r_t[:, 0:w], vv[:, 0:w], FT.Exp,
                                     scale=-0.5, bias=halfc)
                hv = smalls.tile([128, 2 * NKC], F32, name="hv")
                nc.vector.tensor_scalar(hv[:, 0:w], vv[:, 0:w], -0.5, None,
                                        OP.mult)
                cur = r_t
                for it in range(2):
                    b = smalls.tile([128, 2 * NKC], F32, name=f"nb{it}")
                    nc.vector.tensor_tensor(b[:, 0:w], cur[:, 0:w],
                                            cur[:, 0:w], OP.mult)
                    t = smalls.tile([128, 2 * NKC], F32, name=f"nt{it}")
                    nc.vector.scalar_tensor_tensor(
                        t[:, 0:w], b[:, 0:w], 1.0, hv[:, 0:w],
                        OP.mult, OP.mult)
                    nxt = r_t if it == 1 else smalls.tile(
                        [128, 2 * NKC], F32, name=f"nr{it}")
                    nc.vector.scalar_tensor_tensor(
                        nxt[:, 0:w], t[:, 0:w], 1.5, cur[:, 0:w],
                        OP.add, OP.mult)
                    cur = nxt

            batched_r(ssqQ, rQ, NQC)
            batched_r(ssqKV, rKV, 2 * NKC)
            nc.vector.tensor_scalar_mul(rkc, rKV[:, 0:NKC], SCALE_C)

            def q_finish(pq_tr, c):
                tsrc = ln_sb.tile([128, D], BF16, name="tsrc")
                nc.vector.tensor_scalar(tsrc, cpreQ[c], rQ[:, c:c + 1],
                                        None, OP.mult)
                tpb = pq_tr.tile([128, 2 * 128], BF16, name="tpb")
                nc.tensor.transpose(tpb[:, 0:128], tsrc[:, 0:128], idn_s)
                nc.tensor.transpose(tpb[0:64, 128:256], tsrc[:, 128:192],
                                    idn_s)
                s, j = c // 4, c % 4
                jsl = slice(j * 128, (j + 1) * 128)
                if general_gb:
                    nc.vector.tensor_scalar(
                        qT0s[s][:, jsl], tpb[:, 0:128], gca, bca,
                        OP.mult, OP.add)
                    nc.vector.tensor_scalar(
                        qT1s[s][0:64, jsl], tpb[0:64, 128:256], gcb, bcb,
                        OP.mult, OP.add)
                else:
                    nc.vector.tensor_copy(qT0s[s][:, jsl], tpb[:, 0:128])
                    nc.vector.tensor_copy(qT1s[s][0:64, jsl],
                                          tpb[0:64, 128:256])

            def k_finish(pq_tr, c):
                tsrc = ln_sb.tile([128, D], BF16, name="tsrc")
                nc.vector.tensor_scalar(tsrc, cpreK[c], rKV[:, c:c + 1],
                                        None, OP.mult)
                tpb = pq_tr.tile([128, 2 * 128], BF16, name="tpb")
                nc.tensor.transpose(tpb[:, 0:128], tsrc[:, 0:128], idn_s)
                nc.tensor.transpose(tpb[0:64, 128:256], tsrc[:, 128:192],
                                    idn_s)
                t, j = c // 4, c % 4
                jsl = slice(j * 128, (j + 1) * 128)
                nc.vector.tensor_scalar(
                    kT0t[t][:, jsl], tpb[:, 0:128], gca, bca,
                    OP.mult, OP.add)
                nc.vector.tensor_scalar(
                    kT1t[t][0:64, jsl], tpb[0:64, 128:256], gcb, bcb,
                    OP.mult, OP.add)

            def v_finish(c):
                rj = rKV[:, NKC + c:NKC + c + 1]
                if general_gb:
                    t1 = ln_sb.tile([128, D], F32, name="t1")
                    nc.vector.tensor_scalar(t1, cpreV[c], rj, None, OP.mult)
                    t2 = ln_sb.tile([128, D], F32, name="t2")
                    nc.vector.tensor_tensor(t2, t1, gbc_s, OP.mult)
                    nc.vector.tensor_tensor(vatc[c][:, 0:192], t2, bbc_s,
                                            OP.add)
                else:
                    nc.vector.tensor_scalar(vatc[c][:, 0:192], cpreV[c], rj,
                                            None, OP.mult)

            with tc.tile_pool(name="pcs_tr", bufs=2, space="PSUM") as pcs_tr, \
                 tc.tile_pool(name="pcs_sc", bufs=2, space="PSUM") as pcs_sc, \
                 tc.tile_pool(name="pcs_oa", bufs=2, space="PSUM") as pcs_oa, \
                 tc.tile_pool(name="pcs_ob", bufs=2, space="PSUM") as pcs_ob:
                for c in range(4):
                    q_finish(pcs_tr, c)
                if general_gb:
                    k_finish(pcs_tr, 0)
                v_finish(0)

                for s in range(NSLl):
                    qsl = slice(s * SLAB, (s + 1) * SLAB)
                    oA = pcs_oa.tile([128, SLAB], F32, name="oA")
                    oB = pcs_ob.tile([65, SLAB], F32, name="oB")
                    pt_prev = None
                    for c in range(NKC):
                        if s == 0:
                            if c + 4 < NQC:
                                q_finish(pcs_tr, c + 4)
                            if general_gb and c + 1 < NKC:
                                k_finish(pcs_tr, c + 1)
                            if c + 1 < NKC:
                                v_finish(c + 1)
                        t, j = c // 4, c % 4
                        jsl = slice(j * 128, (j + 1) * 128)
                        sct = pcs_sc.tile([128, SLAB], F32, name="sct")
                        nc.tensor.matmul(sct, lhsT=kT0t[t][:, jsl],
                                         rhs=qT0s[s], start=True, stop=False)
                        nc.tensor.matmul(sct, lhsT=kT1t[t][:, jsl],
                                         rhs=qT1s[s], start=False, stop=True)
                        pt = pt_pool.tile([128, SLAB], BF16, name="pt")
                        sc_arg = SCALE_C if general_gb else rkc[:, c:c + 1]
                        nc.scalar.activation(pt, sct, FT.Exp, scale=sc_arg)
                        if pt_prev is not None:
                            cp = c - 1
                            nc.tensor.matmul(oA, lhsT=vatc[cp][:, 0:128],
                                             rhs=pt_prev, start=(cp == 0),
                                             stop=False)
                            nc.tensor.matmul(oB, lhsT=vatc[cp][:, 128:193],
                                             rhs=pt_prev, start=(cp == 0),
                                             stop=False)
                        pt_prev = pt
                    nc.tensor.matmul(oA, lhsT=vatc[NKC - 1][:, 0:128],
                                     rhs=pt_prev, start=False, stop=True)
                    nc.tensor.matmul(oB, lhsT=vatc[NKC - 1][:, 128:193],
                                     rhs=pt_prev, start=False, stop=True)
                    ea = ev.tile([128, SLAB], F32, name="ea")
                    nc.vector.tensor_copy(ea, oA)
                    eb = ev.tile([65, SLAB], F32, name="eb")
                    nc.vector.tensor_copy(eb, oB)
                    nc.sync.dma_start(outa[:, qsl], ea)
                    nc.sync.dma_start(outb[:, qsl], eb)

    nc.compile()
    return nc


def _get_program(general_gb: bool):
    key = bool(general_gb)
    if key not in _PROGRAM_CACHE:
        if key:
            _PROGRAM_CACHE[key] = _build_program_legacy(True)
        else:
            _PROGRAM_CACHE[key] = _build_program_fast()
    return _PROGRAM_CACHE[key]


def _patchify(x):
    # (1, C, IMG, IMG) -> (S, D); token s=(i,j), feature d=(c, wi, wj)
    t = x.reshape(C, NS, WS, NS, WS)
    t = np.transpose(t, (1, 3, 0, 2, 4))
    return np.ascontiguousarray(t.reshape(S, D))


def _unpatchify(tokens):
    # (S, D) -> (1, C, IMG, IMG)
    t = tokens.reshape(NS, NS, C, WS, WS)
    t = np.transpose(t, (2, 0, 3, 1, 4))
    return np.ascontiguousarray(t.reshape(1, C, IMG, IMG))


def _prepare(inputs):
    x = np.asarray(inputs["x"], dtype=np.float32)
    Wq = np.asarray(inputs["Wq"], dtype=np.float32)
    Wk = np.asarray(inputs["Wk"], dtype=np.float32)
    Wv = np.asarray(inputs["Wv"], dtype=np.float32)
    bq = np.asarray(inputs["bq"], dtype=np.float32)
    bk = np.asarray(inputs["bk"], dtype=np.float32)
    bv = np.asarray(inputs["bv"], dtype=np.float32)
    gamma = np.asarray(inputs["gamma"], dtype=np.float32)
    beta = np.asarray(inputs["beta"], dtype=np.float32)

    general_gb = not (np.all(gamma == 1.0) and np.all(beta == 0.0))
    nc = _get_program(general_gb)

    dt = np.float16 if not general_gb else ml_dtypes.bfloat16
    xs = _patchify(x)

    def centered(W, b):
        Wc = W - W.mean(axis=0, keepdims=True)
        bc = b - b.mean()
        return Wc, bc

    Wqc, bqc = centered(Wq, bq)
    Wkc, bkc = centered(Wk, bk)
    Wvc, bvc = centered(Wv, bv)

    wa = np.concatenate([Wqc.T[0:128], Wkc.T[0:128], Wvc.T[0:128]], axis=1)
    wb = np.zeros((128, 3 * D), np.float32)
    wb[0:64, 0:D] = Wqc.T[128:192]
    wb[0:64, D:2 * D] = Wkc.T[128:192]
    wb[0:64, 2 * D:3 * D] = Wvc.T[128:192]
    wb[64, 0:D] = bqc
    wb[64, D:2 * D] = bkc
    wb[64, 2 * D:3 * D] = bvc
    wa = wa.astype(dt)
    wb = wb.astype(dt)
    wka = Wkc.T[0:128].astype(dt)
    wkb = np.zeros((128, D), np.float32)
    wkb[0:64] = Wkc.T[128:192]
    wkb[64] = bkc
    wkb = wkb.astype(dt)
    # wqd: q dims 128:192 duplicated along cols; [:, 0:128] contracts xa,
    # [:, 128:256] contracts xb (rows 0:64 = WqT[128:192], row 64 = bias)
    wqd = np.zeros((128, 256), np.float32)
    wqd[:, 0:64] = Wqc.T[0:128, 128:192]
    wqd[:, 64:128] = Wqc.T[0:128, 128:192]
    wqd[0:64, 128:192] = Wqc.T[128:192, 128:192]
    wqd[0:64, 192:256] = Wqc.T[128:192, 128:192]
    wqd[64, 128:192] = bqc[128:192]
    wqd[64, 192:256] = bqc[128:192]
    wqd = wqd.astype(dt)
    idn = np.eye(128, dtype=dt)

    dithb = (-SHIFT - np.log(_WCOL)).reshape(128, 1).astype(np.float32)
    dithw = _WCOL.reshape(128, 1).astype(np.float32)
    ones8 = _WCOL.reshape(128, 1).astype(ml_dtypes.float8_e4m3)
    # one-hot selector blocks for the r_k row->tile broadcast matmuls:
    # selbc[k, r*128+m] = (k == r)
    selbc = np.zeros((32, SEG), np.float16)
    for r in range(32):
        selbc[r, r * 128:(r + 1) * 128] = 1.0

    in_maps = []
    for core in range(NCORES):
        g, h = core // 2, core % 2
        seg = xs[g * SEG:(g + 1) * SEG]
        perm = np.concatenate(
            [seg[h * NQ:(h + 1) * NQ], seg[(1 - h) * NQ:(2 - h) * NQ]],
            axis=0)
        xsT = perm.T  # (192, 4096)
        xav = np.ascontiguousarray(xsT[0:128]).astype(dt)
        xbv = np.zeros((128, SEG), np.float32)
        xbv[0:64] = xsT[128:192]
        xbv[64] = 1.0
        xbv = xbv.astype(dt)
        im = {"xa": xav, "xb": xbv, "wa": wa, "wb": wb,
              "wka": wka, "wkb": wkb, "idn": idn}
        if not general_gb:
            im["wqd"] = wqd
        if general_gb:
            im["gcol"] = gamma.reshape(D, 1).copy()
            im["bcol"] = beta.reshape(D, 1).copy()
            im["gbc"] = np.broadcast_to(gamma, (128, D)).copy()
            im["bbc"] = np.broadcast_to(beta, (128, D)).copy()
        else:
            im["dithb"] = dithb
            im["dithw"] = dithw
            im["ones8"] = ones8
            im["selbc"] = selbc
        in_maps.append(im)

    return nc, in_maps, general_gb


def _postprocess(res):
    out_tokens = np.empty((S, D), np.float32)
    for core in range(NCORES):
        g, h = core // 2, core % 2
        outa = res.results[core]["outa"]  # (128, NQ) unnormalized outT
        outb = res.results[core]["outb"]  # (65, NQ): 0:64 outT, row 64 sums
        o_t = np.concatenate([outa, outb[0:64]], axis=0)  # (192, NQ)
        sums = outb[64]
        out_tokens[g * SEG + h * NQ: g * SEG + (h + 1) * NQ] = \
            (o_t / sums).T

    return _unpatchify(out_tokens)


def kernel(**inputs):
    nc, in_maps, _ = _prepare(inputs)
    res = run_bass_kernel_spmd(nc, in_maps, list(range(NCORES)))
    return _postprocess(res)

